# revision 1
# baseline (speedup 1.0000x reference)
"""Trainium2 Bass kernel for nn_E1DeepPredictor (attention prologue + 2-layer LSTM rollout).

Strategy: 8-way tensor parallel across NeuronCores.
- All activations are kept feature-major ("transposed": [feat, batch]) so matmuls are
  out[M=feat_tile,128 x N=batch,256] = W_tile[K,128].T @ actT[K,256], with per-partition
  biases folded into ScalarEngine activation ops.
- LSTM gate weights are column(=output hidden unit)-sharded per core: core j owns hidden
  units [128j,128j+128) of both layers (i/f/g/o blocks each). op_W1/op_W2 replicated.
- Per timestep: two 8-rank AllGathers (h0 shard, h1 shard) through DRAM bounce buffers,
  overlapped with the recurrent matmuls that do not depend on them.
- The attention + prior-generator prologue is computed once, replicated on all cores.
- Sharded epilogue matmuls (per-core op_W2 / pg_W2 row shards) produce per-core output
  slices so each core only writes 1/8 of the predictions.
"""

import sys

for _p in ("/opt/trn_rl_repo", "/opt/trn_rl_repo/concourse"):
    if _p not in sys.path:
        sys.path.insert(0, _p)

import numpy as np

import concourse.bass as bass
import concourse.tile as tile
import concourse.mybir as mybir
from concourse import bacc
from concourse import bass_utils

AF = mybir.ActivationFunctionType
F32 = mybir.dt.float32

LATENT = 512
HIDDEN = 1024
SLOTS = 16
BATCH = 256
NCORES = 8
P = 128

HSH = HIDDEN // NCORES          # 128 hidden units per core
LSH = LATENT // NCORES          # 64 latent rows per core (output shard)
N_B = BATCH                     # batch is the matmul free dim (256)
NH = HIDDEN // P                # 8 K-tiles over hidden
NL = LATENT // P                # 4 K-tiles over latent


def _lhsT(W):
    """W [O, K] -> [128, n_m, n_k, 128] with [p, m, k, c] = W[m*128+c, k*128+p]."""
    O, K = W.shape
    n_m, n_k = O // P, K // P
    return np.ascontiguousarray(
        W.reshape(n_m, P, n_k, P).transpose(3, 0, 2, 1).astype(np.float32)
    )


def _ktiles(A_T):
    """A_T [K, N] -> [128, n_k, N]  (K-tiles stacked on a free axis)."""
    K, N = A_T.shape
    return np.ascontiguousarray(
        A_T.reshape(K // P, P, N).transpose(1, 0, 2).astype(np.float32)
    )


def _bias_cols(b):
    """b [O] -> [128, n_m] (column m = bias for feature tile m)."""
    return np.ascontiguousarray(b.reshape(-1, P).T.astype(np.float32))


def _gate_shard(W, j):
    """Take hidden-unit shard j of a [4H, ...] gate matrix -> [4*HSH, ...]."""
    return np.concatenate(
        [W[g * HIDDEN + j * HSH:g * HIDDEN + (j + 1) * HSH] for g in range(4)], axis=0
    )


DEBUG = False


def build_graph(horizon, nonzero_vb):
    nc = bacc.Bacc("TRN2", target_bir_lowering=False, debug=False, num_devices=NCORES)

    def din(name, shape):
        return nc.dram_tensor(name, list(shape), F32, kind="ExternalInput").ap()

    # --- inputs (feature-major, host-pre-tiled) ---
    csT = din("csT", [P, NL, N_B])
    memT = din("memT", [P, NH, SLOTS])
    qW = din("qW", [P, NH, NL, P])
    kW = din("kW", [P, NH, NH, P])
    vWT = din("vWT", [P, NH, HIDDEN])
    moW = din("moW", [P, NL, NH, P])
    pgW1 = din("pgW1", [P, NH, 2 * NL, P])
    pgW2 = din("pgW2", [P, NL, NH, P])
    pg2sh = din("pg2sh", [P, NH, LSH])
    wih0 = din("wih0", [P, 4, NL, P])
    whh0 = din("whh0", [P, 4, NH, P])
    wih1 = din("wih1", [P, 4, NH, P])
    whh1 = din("whh1", [P, 4, NH, P])
    opW1 = din("opW1", [P, NH, NH, P])
    opW2 = din("opW2", [P, NL, NH, P])
    op2sh = din("op2sh", [P, NH, LSH])
    biases = din("biases", [P, 32])
    biases2 = din("biases2", [P, 20])
    bsh = din("bsh", [LSH, 2])
    vb_row = din("v_b_row", [1, HIDDEN]) if nonzero_vb else None

    out_pred = nc.dram_tensor("out_pred", [horizon, LSH, N_B], F32, kind="ExternalOutput").ap()
    out_prior = nc.dram_tensor("out_prior", [LSH, N_B], F32, kind="ExternalOutput").ap()
    dbg = {}
    if DEBUG:
        for nm, shp in [("d_h0sh", [P, N_B]), ("d_h0f", [P, NH, N_B]),
                        ("d_h1sh", [P, N_B]), ("d_h1f", [P, NH, N_B]),
                        ("d_o1", [P, NH, N_B]), ("d_g1i", [P, N_B]),
                        ("d_xT", [P, NL, N_B]), ("d_g0i", [P, N_B])]:
            dbg[nm] = nc.dram_tensor(nm, shp, F32, kind="ExternalOutput").ap()

    # bias column indices
    BQ, BK, BMO, BPG1, BPG2 = 0, 8, 16, 20, 28      # in `biases`
    B0, B1, BO1, BO2 = 0, 4, 8, 16                  # in `biases2`

    RG = [list(range(NCORES))]

    from contextlib import ExitStack
    with tile.TileContext(nc) as tc, ExitStack() as _stack:
        state = _stack.enter_context(tc.tile_pool(name="state", bufs=1))
        psB = _stack.enter_context(tc.tile_pool(name="psB", bufs=1, space="PSUM"))
        dram = _stack.enter_context(tc.tile_pool(name="dram", bufs=2, space="DRAM"))

        # PSUM budget (8 banks): 4 tags x [128,4,256]f32 = 2 banks each.
        def ps_big(tag):
            return psB.tile([P, 4, N_B], F32, tag=tag, name=f"ps_{tag}")

        csT_sb = state.tile([P, NL, N_B], F32, tag="csT")
        nc.sync.dma_start(csT_sb[:], csT[:])
        bias_sb = state.tile([P, 32], F32, tag="bias")
        nc.sync.dma_start(bias_sb[:], biases[:])
        bias2_sb = state.tile([P, 20], F32, tag="bias2")
        nc.sync.dma_start(bias2_sb[:], biases2[:])
        bsh_sb = state.tile([LSH, 2], F32, tag="bsh")
        nc.sync.dma_start(bsh_sb[:], bsh[:])
        # x state (latent input to layer 0), lives across prologue + all steps
        xT_sb = state.tile([P, NL, N_B], F32, tag="xT", bufs=2)

        # ================= PROLOGUE =================
        with (
            tc.tile_pool(name="pw", bufs=1) as pw,
            tc.tile_pool(name="pact", bufs=2) as pact,
        ):
            ident = pw.tile([P, P], F32, tag="ident")
            from concourse.masks import make_identity
            make_identity(nc, ident[:])

            memT_sb = pw.tile([P, NH, SLOTS], F32, tag="memT")
            nc.sync.dma_start(memT_sb[:], memT[:])

            # qT [1024, 256]
            qW_sb = pw.tile([P, NH, NL, P], F32, tag="qW")
            nc.sync.dma_start(qW_sb[:], qW[:])
            qT_sb = pw.tile([P, NH, N_B], F32, tag="qT")
            for m in range(NH):
                ps = ps_big("g0" if m % 2 == 0 else "g1")
                for k in range(NL):
                    nc.tensor.matmul(ps[:, 0, :], qW_sb[:, m, k, :], csT_sb[:, k, :],
                                     start=(k == 0), stop=(k == NL - 1))
                nc.scalar.activation(qT_sb[:, m, :], ps[:, 0, :], AF.Identity,
                                     bias=bias_sb[:, BQ + m:BQ + m + 1])

            # kT [1024, 16]
            kW_sb = pw.tile([P, NH, NH, P], F32, tag="kW")
            nc.sync.dma_start(kW_sb[:], kW[:])
            kT_sb = pw.tile([P, NH, SLOTS], F32, tag="kT")
            for m in range(NH):
                ps = ps_big("g0" if m % 2 == 0 else "g1")
                for k in range(NH):
                    nc.tensor.matmul(ps[:, 0, 0:SLOTS], kW_sb[:, m, k, :], memT_sb[:, k, :],
                                     start=(k == 0), stop=(k == NH - 1))
                nc.scalar.activation(kT_sb[:, m, :], ps[:, 0, 0:SLOTS], AF.Identity,
                                     bias=bias_sb[:, BK + m:BK + m + 1])

            # v [16, 1024] natural layout
            vWT_sb = pw.tile([P, NH, HIDDEN], F32, tag="vWT")
            nc.sync.dma_start(vWT_sb[:], vWT[:])
            v_sb = pw.tile([SLOTS, HIDDEN], F32, tag="v")
            if nonzero_vb:
                ones16 = pw.tile([1, SLOTS], F32, tag="ones16")
                nc.vector.memset(ones16[:], 1.0)
                vb_sb = pw.tile([1, HIDDEN], F32, tag="vb")
                nc.sync.dma_start(vb_sb[:], vb_row[:])
            for half in range(2):
                ps = psB.tile([SLOTS, 4, N_B], F32, tag="o1ps", name="ps_v")
                psv = ps.rearrange("s a b -> s (a b)")[:, 0:HIDDEN // 2]
                for k in range(NH):
                    nc.tensor.matmul(
                        psv, memT_sb[:, k, :],
                        vWT_sb[:, k, half * (HIDDEN // 2):(half + 1) * (HIDDEN // 2)],
                        start=(k == 0), stop=(k == NH - 1 and not nonzero_vb))
                if nonzero_vb:
                    nc.tensor.matmul(
                        psv, ones16[:],
                        vb_sb[:, half * (HIDDEN // 2):(half + 1) * (HIDDEN // 2)],
                        start=False, stop=True)
                nc.scalar.copy(v_sb[:, half * (HIDDEN // 2):(half + 1) * (HIDDEN // 2)], psv)

            # scores [256, 16] batch-major + softmax along free dim
            w_sb = pw.tile([P, 2, SLOTS], F32, tag="wgt")
            for m in range(2):
                ps = ps_big("g0" if m % 2 == 0 else "g1")
                pss = ps[:, 0, 0:SLOTS]
                for k in range(NH):
                    nc.tensor.matmul(pss, qT_sb[:, k, m * P:(m + 1) * P], kT_sb[:, k, :],
                                     start=(k == 0), stop=(k == NH - 1))
                sc = pact.tile([P, SLOTS], F32, tag="sm_sc")
                nc.scalar.activation(sc[:], pss, AF.Copy, scale=1.0 / float(np.sqrt(HIDDEN)))
                mx = pact.tile([P, 1], F32, tag="sm_mx")
                nc.vector.reduce_max(mx[:], sc[:], axis=mybir.AxisListType.X)
                nmx = pact.tile([P, 1], F32, tag="sm_nmx")
                nc.scalar.mul(nmx[:], mx[:], -1.0)
                ex = pact.tile([P, SLOTS], F32, tag="sm_ex")
                nc.scalar.activation(ex[:], sc[:], AF.Exp, bias=nmx[:])
                sm = pact.tile([P, 1], F32, tag="sm_sum")
                nc.vector.reduce_sum(sm[:], ex[:], axis=mybir.AxisListType.X)
                rs = pact.tile([P, 1], F32, tag="sm_rs")
                nc.vector.reciprocal(rs[:], sm[:])
                nc.vector.tensor_scalar_mul(w_sb[:, m, :], ex[:], rs[:])

            # transpose weights -> wT [16, 256]
            wT_sb = pw.tile([SLOTS, N_B], F32, tag="wT")
            for m in range(2):
                pst = psB.tile([SLOTS, 4, N_B], F32, tag="o1ps", name="ps_t")
                nc.tensor.transpose(pst[:, 0, 0:P], w_sb[:, m, :], ident[:])
                nc.scalar.copy(wT_sb[:, m * P:(m + 1) * P], pst[:, 0, 0:P])

            # ctxT [1024, 256] = (weights @ v).T : lhsT = v slices [16, 128]
            ctxT_sb = pw.tile([P, NH, N_B], F32, tag="ctxT")
            for m in range(NH):
                ps = ps_big("g0" if m % 2 == 0 else "g1")
                nc.tensor.matmul(ps[:, 0, :], v_sb[:, m * P:(m + 1) * P], wT_sb[:],
                                 start=True, stop=True)
                nc.scalar.copy(ctxT_sb[:, m, :], ps[:, 0, :])

            # contextT [512, 256]
            moW_sb = pw.tile([P, NL, NH, P], F32, tag="moW")
            nc.sync.dma_start(moW_sb[:], moW[:])
            conT_sb = pw.tile([P, NL, N_B], F32, tag="conT")
            for m in range(NL):
                ps = ps_big("g0" if m % 2 == 0 else "g1")
                for k in range(NH):
                    nc.tensor.matmul(ps[:, 0, :], moW_sb[:, m, k, :], ctxT_sb[:, k, :],
                                     start=(k == 0), stop=(k == NH - 1))
                nc.scalar.activation(conT_sb[:, m, :], ps[:, 0, :], AF.Identity,
                                     bias=bias_sb[:, BMO + m:BMO + m + 1])

            # prior generator: combined = [cs | context] as K-tiles 0..3 / 4..7
            pgW1_sb = pw.tile([P, NH, 2 * NL, P], F32, tag="pgW1")
            nc.sync.dma_start(pgW1_sb[:], pgW1[:])
            pghT_sb = pw.tile([P, NH, N_B], F32, tag="pghT")
            for m in range(NH):
                ps = ps_big("g0" if m % 2 == 0 else "g1")
                for k in range(2 * NL):
                    rhs = csT_sb[:, k, :] if k < NL else conT_sb[:, k - NL, :]
                    nc.tensor.matmul(ps[:, 0, :], pgW1_sb[:, m, k, :], rhs,
                                     start=(k == 0), stop=(k == 2 * NL - 1))
                nc.scalar.activation(pghT_sb[:, m, :], ps[:, 0, :], AF.Relu,
                                     bias=bias_sb[:, BPG1 + m:BPG1 + m + 1])

            pgW2_sb = pw.tile([P, NL, NH, P], F32, tag="pgW2")
            nc.sync.dma_start(pgW2_sb[:], pgW2[:])
            for m in range(NL):
                ps = ps_big("g0" if m % 2 == 0 else "g1")
                for k in range(NH):
                    nc.tensor.matmul(ps[:, 0, :], pgW2_sb[:, m, k, :], pghT_sb[:, k, :],
                                     start=(k == 0), stop=(k == NH - 1))
                nc.scalar.activation(xT_sb[:, m, :], ps[:, 0, :], AF.Identity,
                                     bias=bias_sb[:, BPG2 + m:BPG2 + m + 1])

            # sharded prior output rows
            pg2sh_sb = pw.tile([P, NH, LSH], F32, tag="pg2sh")
            nc.sync.dma_start(pg2sh_sb[:], pg2sh[:])
            pssh = psB.tile([LSH, 4, N_B], F32, tag="prps", name="ps_prior")
            for k in range(NH):
                nc.tensor.matmul(pssh[:, 0, :], pg2sh_sb[:, k, :], pghT_sb[:, k, :],
                                 start=(k == 0), stop=(k == NH - 1))
            prior_sb = pact.tile([LSH, N_B], F32, tag="prior")
            nc.scalar.activation(prior_sb[:], pssh[:, 0, :], AF.Identity, bias=bsh_sb[:, 0:1])
            nc.sync.dma_start(out_prior[:], prior_sb[:])

        # ================= LSTM =================
        with (
            tc.tile_pool(name="wts", bufs=1) as wts,
            tc.tile_pool(name="act", bufs=2) as act,
            tc.tile_pool(name="act1", bufs=1) as act1,
        ):
            wih0_sb = wts.tile([P, 4, NL, P], F32, tag="wih0")
            nc.sync.dma_start(wih0_sb[:], wih0[:])
            whh0_sb = wts.tile([P, 4, NH, P], F32, tag="whh0")
            nc.sync.dma_start(whh0_sb[:], whh0[:])
            wih1_sb = wts.tile([P, 4, NH, P], F32, tag="wih1")
            nc.sync.dma_start(wih1_sb[:], wih1[:])
            whh1_sb = wts.tile([P, 4, NH, P], F32, tag="whh1")
            nc.sync.dma_start(whh1_sb[:], whh1[:])
            opW1_sb = wts.tile([P, NH, NH, P], F32, tag="opW1")
            nc.sync.dma_start(opW1_sb[:], opW1[:])
            opW2_sb = wts.tile([P, NL, NH, P], F32, tag="opW2")
            nc.sync.dma_start(opW2_sb[:], opW2[:])
            op2sh_sb = wts.tile([P, NH, LSH], F32, tag="op2sh")
            nc.sync.dma_start(op2sh_sb[:], op2sh[:])

            def lstm_elt(g_ps, c_old, first, layer):
                """gates psum [128, 4, 256] -> (h_shard, c_new) SBUF [128, 256]."""
                bb = B0 if layer == 0 else B1
                i_t = act.tile([P, N_B], F32, tag=f"i{layer}", name=f"i{layer}")
                nc.scalar.activation(i_t[:], g_ps[:, 0, :], AF.Sigmoid,
                                     bias=bias2_sb[:, bb + 0:bb + 1])
                g_t = act.tile([P, N_B], F32, tag=f"g{layer}", name=f"g{layer}")
                nc.scalar.activation(g_t[:], g_ps[:, 2, :], AF.Tanh,
                                     bias=bias2_sb[:, bb + 2:bb + 3])
                o_t = act.tile([P, N_B], F32, tag=f"o{layer}", name=f"o{layer}")
                nc.scalar.activation(o_t[:], g_ps[:, 3, :], AF.Sigmoid,
                                     bias=bias2_sb[:, bb + 3:bb + 4])
                c_new = act.tile([P, N_B], F32, tag=f"c{layer}", name=f"c{layer}")
                if first:
                    nc.vector.tensor_mul(c_new[:], i_t[:], g_t[:])
                else:
                    f_t = act.tile([P, N_B], F32, tag=f"f{layer}", name=f"f{layer}")
                    nc.scalar.activation(f_t[:], g_ps[:, 1, :], AF.Sigmoid,
                                         bias=bias2_sb[:, bb + 1:bb + 2])
                    ig = act.tile([P, N_B], F32, tag=f"ig{layer}", name=f"ig{layer}")
                    nc.vector.tensor_mul(ig[:], i_t[:], g_t[:])
                    fc = act.tile([P, N_B], F32, tag=f"fc{layer}", name=f"fc{layer}")
                    nc.vector.tensor_mul(fc[:], f_t[:], c_old[:])
                    nc.vector.tensor_add(c_new[:], ig[:], fc[:])
                tc_t = act.tile([P, N_B], F32, tag=f"tc{layer}", name=f"tc{layer}")
                nc.scalar.activation(tc_t[:], c_new[:], AF.Tanh)
                h_sh = act.tile([P, N_B], F32, tag=f"hsh{layer}", name=f"hsh{layer}")
                nc.vector.tensor_mul(h_sh[:], o_t[:], tc_t[:])
                return h_sh, c_new

            c0_t = c1_t = None
            h0f = h1f = None
            g0_ps = None

            for t in range(horizon):
                first = t == 0
                # --- gates0: ih part (hh part emitted at end of prev step) ---
                if first:
                    g0_ps = ps_big("g0")
                for k in range(NL):
                    for m in range(4):
                        nc.tensor.matmul(g0_ps[:, m, :], wih0_sb[:, m, k, :], xT_sb[:, k, :],
                                         start=(first and k == 0), stop=(k == NL - 1))
                if DEBUG and first:
                    xTd = act.tile([P, NL, N_B], F32, tag="dbg_xT", name="dbg_xT")
                    nc.vector.tensor_copy(xTd[:], xT_sb[:])
                    nc.sync.dma_start(dbg["d_xT"][:], xTd[:])
                    g0id = act.tile([P, N_B], F32, tag="dbg_g0i", name="dbg_g0i")
                    nc.vector.tensor_copy(g0id[:], g0_ps[:, 0, :])
                    nc.sync.dma_start(dbg["d_g0i"][:], g0id[:])
                h0_sh, c0_t = lstm_elt(g0_ps, c0_t, first, 0)

                # --- AG #1 (h0); overlap: hh1 matmuls from previous h1 ---
                in_b0 = dram.tile([P, N_B], F32, tag="ag0in", name="ag0in")
                nc.sync.dma_start(in_b0[:], h0_sh[:])
                out_b0 = dram.tile([NCORES * P, N_B], F32, tag="ag0out", name="ag0out")
                nc.gpsimd.collective_compute(
                    "AllGather", mybir.AluOpType.bypass, replica_groups=RG,
                    ins=[in_b0.opt()], outs=[out_b0.opt()])

                g1_ps = ps_big("g1")
                if not first:
                    for k in range(NH):
                        for m in range(4):
                            nc.tensor.matmul(g1_ps[:, m, :], whh1_sb[:, m, k, :], h1f[:, k, :],
                                             start=(k == 0), stop=False)

                h0f = act.tile([P, NH, N_B], F32, tag="h0full", name="h0full")
                nc.sync.dma_start(h0f[:], out_b0[:].rearrange("(kt p) b -> p kt b", p=P))

                # --- gates1: ih part from gathered h0 ---
                for k in range(NH):
                    for m in range(4):
                        nc.tensor.matmul(g1_ps[:, m, :], wih1_sb[:, m, k, :], h0f[:, k, :],
                                         start=(first and k == 0), stop=(k == NH - 1))
                h1_sh, c1_t = lstm_elt(g1_ps, c1_t, first, 1)
                if DEBUG and first:
                    g1i = act.tile([P, N_B], F32, tag="dbg_g1i", name="dbg_g1i")
                    nc.vector.tensor_copy(g1i[:], g1_ps[:, 0, :])
                    nc.sync.dma_start(dbg["d_g1i"][:], g1i[:])
                    nc.sync.dma_start(dbg["d_h0sh"][:], h0_sh[:])
                    nc.sync.dma_start(dbg["d_h0f"][:], h0f[:])
                    nc.sync.dma_start(dbg["d_h1sh"][:], h1_sh[:])

                # --- AG #2 (h1); overlap: next step's hh0 matmuls ---
                in_b1 = dram.tile([P, N_B], F32, tag="ag1in", name="ag1in")
                nc.sync.dma_start(in_b1[:], h1_sh[:])
                out_b1 = dram.tile([NCORES * P, N_B], F32, tag="ag1out", name="ag1out")
                nc.gpsimd.collective_compute(
                    "AllGather", mybir.AluOpType.bypass, replica_groups=RG,
                    ins=[in_b1.opt()], outs=[out_b1.opt()])

                if t + 1 < horizon:
                    g0_ps = ps_big("g0")
                    for k in range(NH):
                        for m in range(4):
                            nc.tensor.matmul(g0_ps[:, m, :], whh0_sb[:, m, k, :], h0f[:, k, :],
                                             start=(k == 0), stop=False)

                h1f = act.tile([P, NH, N_B], F32, tag="h1full", name="h1full")
                nc.sync.dma_start(h1f[:], out_b1[:].rearrange("(kt p) b -> p kt b", p=P))
                if DEBUG and first:
                    nc.sync.dma_start(dbg["d_h1f"][:], h1f[:])

                # --- output stack ---
                o1_sb = act1.tile([P, NH, N_B], F32, tag="o1", name="o1")
                for half in range(2):
                    oh_ps = psB.tile([P, 4, N_B], F32, tag="o1ps", name="ps_o1")
                    for k in range(NH):
                        for m in range(4):
                            nc.tensor.matmul(oh_ps[:, m, :], opW1_sb[:, half * 4 + m, k, :],
                                             h1f[:, k, :], start=(k == 0), stop=(k == NH - 1))
                    for m in range(4):
                        mm = half * 4 + m
                        nc.scalar.activation(o1_sb[:, mm, :], oh_ps[:, m, :], AF.Relu,
                                             bias=bias2_sb[:, BO1 + mm:BO1 + mm + 1])

                if DEBUG and first:
                    nc.sync.dma_start(dbg["d_o1"][:], o1_sb[:])
                if t + 1 < horizon:
                    pr_ps = psB.tile([P, 4, N_B], F32, tag="prps", name="ps_pred")
                    for k in range(NH):
                        for m in range(NL):
                            nc.tensor.matmul(pr_ps[:, m, :], opW2_sb[:, m, k, :], o1_sb[:, k, :],
                                             start=(k == 0), stop=(k == NH - 1))
                    xT_next = state.tile([P, NL, N_B], F32, tag="xT", bufs=2, name="xT")
                    for m in range(NL):
                        nc.scalar.activation(xT_next[:, m, :], pr_ps[:, m, :], AF.Identity,
                                             bias=bias2_sb[:, BO2 + m:BO2 + m + 1])
                    xT_sb = xT_next

                # sharded pred rows for output
                psh = psB.tile([LSH, 4, N_B], F32, tag="o1ps", name="ps_predsh")
                for k in range(NH):
                    nc.tensor.matmul(psh[:, 0, :], op2sh_sb[:, k, :], o1_sb[:, k, :],
                                     start=(k == 0), stop=(k == NH - 1))
                pred_sh = act.tile([LSH, N_B], F32, tag="predshs", name="pred_sh")
                nc.scalar.activation(pred_sh[:], psh[:, 0, :], AF.Identity, bias=bsh_sb[:, 1:2])
                nc.sync.dma_start(out_pred[t], pred_sh[:])

    nc.compile()
    return nc


_CACHE = {}


def _get_graph(horizon, nonzero_vb):
    key = (horizon, nonzero_vb)
    if key not in _CACHE:
        _CACHE[key] = build_graph(horizon, nonzero_vb)
    return _CACHE[key]


def kernel(current_state, horizon, memory,
           q_W, q_b, k_W, k_b, v_W, v_b, mo_W, mo_b,
           pg_W1, pg_b1, pg_W2, pg_b2,
           w_ih0, w_hh0, b_ih0, b_hh0,
           w_ih1, w_hh1, b_ih1, b_hh1,
           op_W1, op_b1, op_W2, op_b2):
    T = int(horizon)
    f = lambda a: np.asarray(a, dtype=np.float32)
    current_state = f(current_state)
    memory = f(memory)

    b0 = f(b_ih0) + f(b_hh0)
    b1 = f(b_ih1) + f(b_hh1)

    nonzero_vb = bool(np.any(f(v_b)))
    nc = _get_graph(T, nonzero_vb)

    shared = {
        "csT": _ktiles(current_state.T),
        "memT": _ktiles(memory.T),
        "qW": _lhsT(f(q_W)),
        "kW": _lhsT(f(k_W)),
        "vWT": _ktiles(f(v_W).T),
        "moW": _lhsT(f(mo_W)),
        "pgW1": _lhsT(f(pg_W1)),
        "pgW2": _lhsT(f(pg_W2)),
        "opW1": _lhsT(f(op_W1)),
        "opW2": _lhsT(f(op_W2)),
    }
    biases = np.zeros((P, 32), np.float32)
    biases[:, 0:8] = _bias_cols(f(q_b))
    biases[:, 8:16] = _bias_cols(f(k_b))
    biases[:, 16:20] = _bias_cols(f(mo_b))
    biases[:, 20:28] = _bias_cols(f(pg_b1))
    biases[:, 28:32] = _bias_cols(f(pg_b2))
    shared["biases"] = biases
    if nonzero_vb:
        shared["v_b_row"] = f(v_b).reshape(1, HIDDEN)

    in_maps = []
    for j in range(NCORES):
        m = dict(shared)
        m["wih0"] = _lhsT(_gate_shard(f(w_ih0), j))
        m["whh0"] = _lhsT(_gate_shard(f(w_hh0), j))
        m["wih1"] = _lhsT(_gate_shard(f(w_ih1), j))
        m["whh1"] = _lhsT(_gate_shard(f(w_hh1), j))
        b2 = np.zeros((P, 20), np.float32)
        b2[:, 0:4] = _bias_cols(_gate_shard(b0[:, None], j)[:, 0])
        b2[:, 4:8] = _bias_cols(_gate_shard(b1[:, None], j)[:, 0])
        b2[:, 8:16] = _bias_cols(f(op_b1))
        b2[:, 16:20] = _bias_cols(f(op_b2))
        m["biases2"] = b2
        rows = slice(j * LSH, (j + 1) * LSH)
        m["op2sh"] = _ktiles(f(op_W2)[rows].T)
        m["pg2sh"] = _ktiles(f(pg_W2)[rows].T)
        bshv = np.zeros((LSH, 2), np.float32)
        bshv[:, 0] = f(pg_b2)[rows]
        bshv[:, 1] = f(op_b2)[rows]
        m["bsh"] = bshv
        in_maps.append(m)

    res = bass_utils.run_bass_kernel_spmd(nc, in_maps, core_ids=list(range(NCORES)))

    predT = np.concatenate([res.results[j]["out_pred"] for j in range(NCORES)], axis=1)
    prior_T = np.concatenate([res.results[j]["out_prior"] for j in range(NCORES)], axis=0)
    predictions = np.ascontiguousarray(predT.transpose(2, 0, 1))   # [256, T, 512]
    prior = np.ascontiguousarray(prior_T.T)                        # [256, 512]
    return predictions, prior
